# revision 12
# baseline (speedup 1.0000x reference)
"""Trainium2 Bass kernel for nn_NeuralDecisionTree.

Math (per sample b):
  h[b,f,i] = x[b,f] * W[i] + bias[f,i],   W = [1,2,3,4],
  bias[f,:] = cumsum([0, -sort(cut_points[f])])           (f=0..7, i=0..3)
  leaf[b, i0..i7] = prod_f h[b,f,i_f]                      (65536-wide kron)
  out[b,c] = sum_leaf leaf[b,leaf] * leaf_score[leaf,c]    (c=0..9)

Kernel strategy (pure batch-data-parallel over 8 cores, 256 rows each):
  W is folded into leaf_score on the host (h' = x + bias/W;
  LS' = leaf_score * kron(W,..,W)), so the device math is
  out[b,c] = sum_u A[b,u] * R[b,u,c],  R[b,:,:] = Bv[b,:] @ LSs,
  A = kron(h'0..h'2) [B,64], Bv = kron(h'3..h'7) [B,1024],
  LSs[v, c*64+u] = LS'[u*1024+v, c]  (host-prepped, fp32r-rounded, replicated).
  Per 128-row tile: kron via broadcast tensor_tensor ops (DVE/ACT),
  transpose Bv via TensorE into 2 packed PSUM banks, contract with LSs on
  TensorE (fp32r), final A-weighted segmented reduce on DVE.
"""

import os
import sys

sys.path.insert(0, "/opt/trn_rl_repo")

import numpy as np

import concourse.bass as bass
from concourse import bacc
import concourse.mybir as mybir
import concourse.tile as tile
from concourse.bass_utils import run_bass_kernel_spmd

F32 = mybir.dt.float32
F32R = mybir.dt.float32r

N_CORES = 8
BATCH = 2048
ROWS_PER_CORE = BATCH // N_CORES  # 256
TILES_PER_CORE = ROWS_PER_CORE // 128  # 2
NF = 8          # features
NB = 4          # bins per feature (D+1)
NC_OUT = 10     # classes
U = 64          # kron(feat 0,1,2)
V = 1024        # kron(feat 3..7)
VCHUNKS = V // 128  # 8
NCOL = NC_OUT * U   # 640 columns of LSs, layout c*64+u
NHALF = NCOL // 2   # 320 (two PSUM tiles per chunk-matmul)
LSDMA = 4           # number of chunked ls DMAs (2 v-chunks each)
XBC = TILES_PER_CORE * NF + NF * NB  # x | bias cols

LAST_RESULT = None  # BassKernelResults of the most recent run (for test.py)


def _build_nc():
    nc = bacc.Bacc("TRN2", target_bir_lowering=False, debug=False,
                   num_devices=N_CORES)
    xb_in = nc.declare_dram_parameter("xb", [128, XBC], F32, isOutput=False)
    id_in = nc.declare_dram_parameter("ident", [128, 128], F32R, isOutput=False)
    ls_in = nc.declare_dram_parameter("ls", [128, VCHUNKS * NCOL], F32R, isOutput=False)
    out_ext = nc.declare_dram_parameter("out", [ROWS_PER_CORE, NC_OUT], F32, isOutput=True)

    with tile.TileContext(nc) as tc:
        with (
            tc.tile_pool(name="consts", bufs=1) as consts,
            tc.tile_pool(name="work", bufs=2) as work,
            tc.tile_pool(name="bt", bufs=2) as btp,
            tc.tile_pool(name="tpsum", bufs=4, space="PSUM") as tpsum,
            tc.tile_pool(name="rpsum", bufs=2, space="PSUM") as rpsum,
        ):
            # Head DMAs (x|bias, then identity) issued FIRST on the same
            # HWDGE ring as the ls stream: FIFO guarantees they land before
            # the 2.6MB ls flood instead of starving behind it.
            xb = consts.tile([128, XBC], F32)
            nc.sync.dma_start(out=xb[:], in_=xb_in[:])
            ident = consts.tile([128, 128], F32R)
            nc.sync.dma_start(out=ident[:], in_=id_in[:])
            xa = xb[:, 0:TILES_PER_CORE * NF]
            bb = xb[:, TILES_PER_CORE * NF:XBC]

            lst = []
            for j in range(LSDMA):
                lsj = consts.tile([128, (VCHUNKS // LSDMA) * NCOL], F32R, tag=f"ls{j}")
                sl = bass.ts(j, (VCHUNKS // LSDMA) * NCOL)
                nc.sync.dma_start(out=lsj[:], in_=ls_in[:, sl])
                lst.append(lsj)

            def ls_chunk(k, half):
                j, r = divmod(k, VCHUNKS // LSDMA)
                base = r * NCOL + half * NHALF
                return lst[j][:, base:base + NHALF]

            oa = consts.tile([128, TILES_PER_CORE * NC_OUT], F32)

            def bcast0(ap, i, shape):
                return ap.unsqueeze(i).broadcast_to(shape)

            bts, avs, pss = [], [], []
            for t in range(TILES_PER_CORE):
                # h'[:, f*4+i] = x[:, f] + bias[f,i]/W[i]
                h = work.tile([128, NF * NB], F32, tag="h")
                nc.vector.tensor_add(
                    h[:].rearrange("p (f i) -> p f i", f=NF),
                    bcast0(xa[:, t * NF:(t + 1) * NF], 2, [128, NF, NB]),
                    bb[:].rearrange("p (f i) -> p f i", f=NF),
                )

                def hcols(f):
                    return h[:, f * NB:(f + 1) * NB]

                def kron_step(out_t, width, prev, f):
                    # out[:, i*width+s] = prev[:, s] * h'[:, f*4+i]
                    nc.vector.tensor_mul(
                        out_t[:].rearrange("p (i s) -> p i s", i=NB),
                        bcast0(prev[:], 1, [128, NB, width]),
                        bcast0(hcols(f), 2, [128, NB, width]),
                    )

                # A = kron(h0, h1, h2): A[:, i0*16 + i1*4 + i2]
                a1 = work.tile([128, 16], F32, tag="a1")
                kron_step(a1, 4, hcols(2), 1)
                a = work.tile([128, U], F32, tag="a")
                kron_step(a, 16, a1, 0)
                avs.append(a)

                # Bv = kron(h3..h7): Bv[:, i3*256 + i4*64 + i5*16 + i6*4 + i7]
                b1 = work.tile([128, 16], F32, tag="b1")
                kron_step(b1, 4, hcols(7), 6)
                b2 = work.tile([128, 64], F32, tag="b2")
                kron_step(b2, 16, b1, 5)
                b3 = work.tile([128, 256], F32, tag="b3")
                kron_step(b3, 64, b2, 4)
                b4 = work.tile([128, V], F32R, tag="b4")
                # last level split: halves on DVE, halves on ACT
                nc.vector.tensor_mul(
                    b4[:, 0:512].rearrange("p (i s) -> p i s", i=2),
                    bcast0(b3[:], 1, [128, 2, 256]),
                    bcast0(h[:, 3 * NB:3 * NB + 2], 2, [128, 2, 256]),
                )
                for j in range(2):
                    nc.scalar.mul(
                        b4[:, 512 + j * 256:512 + (j + 1) * 256], b3[:],
                        h[:, 3 * NB + 2 + j:3 * NB + 3 + j],
                    )

                # Transpose Bv -> BT via TensorE; 4 chunk-transposes per
                # PSUM bank, evacuated with one wide ACT copy each.
                bt = btp.tile([128, V], F32R, tag="btile")
                for q in range(2):
                    tp = tpsum.tile([128, 512], F32R, tag="tp")
                    for j in range(4):
                        k = q * 4 + j
                        nc.tensor.transpose(
                            tp[:, j * 128:(j + 1) * 128],
                            b4[:, k * 128:(k + 1) * 128], ident[:],
                        )
                    nc.scalar.copy(bt[:, q * 512:(q + 1) * 512], tp[:])
                bts.append(bt)
                pss.append((
                    rpsum.tile([128, NHALF], F32, tag="ps0", name=f"ps0_{t}"),
                    rpsum.tile([128, NHALF], F32, tag="ps1", name=f"ps1_{t}"),
                ))

            # R[b, c*64+u] = sum_v Bv[b,v] * LSs[v, c*64+u]  (fp32r),
            # tiles interleaved per v-chunk so matmuls track the ls stream.
            for k in range(VCHUNKS):
                for t in range(TILES_PER_CORE):
                    lhsT = bts[t][:, k * 128:(k + 1) * 128]
                    for half in range(2):
                        nc.tensor.matmul(
                            pss[t][half][:], lhsT, ls_chunk(k, half),
                            start=(k == 0), stop=(k == VCHUNKS - 1),
                        )

            # out[b, c] = sum_u A[b,u] * R[b, c*64+u], per psum half
            for t in range(TILES_PER_CORE):
                abc = bcast0(avs[t][:], 1, [128, NC_OUT // 2, U])
                for half in range(2):
                    tt = work.tile([128, NHALF], F32, tag="tt")
                    nc.vector.tensor_mul(
                        tt[:].rearrange("p (c u) -> p c u", u=U),
                        pss[t][half][:].rearrange("p (c u) -> p c u", u=U),
                        abc,
                    )
                    nc.vector.reduce_sum(
                        oa[:, t * NC_OUT + half * 5:t * NC_OUT + (half + 1) * 5],
                        tt[:].rearrange("p (c u) -> p c u", u=U),
                        axis=mybir.AxisListType.X,
                    )

            nc.scalar.dma_start(
                out=out_ext[:].rearrange("(t p) c -> p t c", p=128),
                in_=oa[:].rearrange("p (t c) -> p t c", c=NC_OUT),
            )

    nc.compile()
    return nc


_NC_CACHE = None


def _install_profiling():
    """Register the axon NTFF profile hook that this image's `antenv` lacks,
    so run_bass_kernel_spmd(trace=True) can measure HW exec time."""
    import types

    try:
        import antenv.axon_hooks  # noqa: F401
        return True
    except ImportError:
        pass
    try:
        from trn_agent_boot.trn_boot import _ntff_profile_via_ctypes
        import antenv

        hook = _ntff_profile_via_ctypes("/opt/axon/libaxon_pjrt.so")
        if hook is None:
            return False
        mod = types.ModuleType("antenv.axon_hooks")
        mod._hook = hook
        mod.set_axon_ntff_profile_hook = lambda h: setattr(mod, "_hook", h)
        mod.get_axon_ntff_profile_hook = lambda: mod._hook
        sys.modules["antenv.axon_hooks"] = mod
        antenv.axon_hooks = mod

        # Artifact upload reaches for a remote bucket; keep everything local.
        import concourse.bass_utils as bu

        bu.upload_artifacts = lambda tmpdir: "local://" + str(tmpdir)
        return True
    except Exception as e:  # pragma: no cover - best effort
        print(f"profiling hook install failed: {e!r}", file=sys.stderr)
        return False


def _to_fp32r(a):
    """Round fp32 to the PE's fp32r format: mantissa truncated to 11 bits (RNE)."""
    u = np.ascontiguousarray(np.asarray(a, np.float32)).view(np.uint32)
    low = u & np.uint32(0xFFF)
    base = u & np.uint32(0xFFFFF000)
    add = (low > 0x800) | ((low == 0x800) & (((u >> np.uint32(12)) & np.uint32(1)) == 1))
    out = base + np.where(add, np.uint32(0x1000), np.uint32(0))
    return out.view(np.float32)


def _host_prep(cut_points, leaf_score):
    W = np.arange(1.0, NB + 1.0, dtype=np.float32)               # [4]
    cp = np.sort(cut_points.astype(np.float32), axis=-1)          # [8,3]
    bias = np.cumsum(
        np.concatenate([np.zeros((NF, 1), np.float32), -cp], axis=1), axis=1
    )                                                             # [8,4]
    # W folded into leaf_score: h' = x + bias/W, LS' = LS * kron(W,...,W)
    bb = np.tile((bias / W[None, :])[None, :, :], (128, 1, 1)).reshape(128, NF * NB)
    wk = np.array([1.0], dtype=np.float64)
    for _ in range(NF):
        wk = np.kron(wk, W.astype(np.float64))                    # [65536]
    lsw = (leaf_score.astype(np.float64) * wk[:, None]).astype(np.float32)
    # LSs[p, k, c, u] = LS'[u*1024 + k*128 + p, c]
    ls4 = lsw.reshape(U, VCHUNKS, 128, NC_OUT)
    lss = np.ascontiguousarray(ls4.transpose(2, 1, 3, 0)).reshape(128, VCHUNKS * NCOL)
    lss = _to_fp32r(lss)
    ident = np.eye(128, dtype=np.float32)
    return bb, lss, ident


def _make_xb(x_shard, bb):
    head = np.empty((128, XBC), dtype=np.float32)
    nx = TILES_PER_CORE * NF
    head[:, 0:nx] = x_shard.reshape(TILES_PER_CORE, 128, NF).transpose(1, 0, 2).reshape(128, nx)
    head[:, nx:nx + NF * NB] = bb
    return head


def kernel(x, cut_points, leaf_score):
    global _NC_CACHE, LAST_RESULT
    x = np.ascontiguousarray(x, dtype=np.float32)
    bb, lss, ident = _host_prep(np.asarray(cut_points), np.asarray(leaf_score))
    if _NC_CACHE is None:
        _NC_CACHE = _build_nc()
    nc = _NC_CACHE

    in_maps = []
    for i in range(N_CORES):
        xs = x[i * ROWS_PER_CORE:(i + 1) * ROWS_PER_CORE]
        in_maps.append({"xb": _make_xb(xs, bb), "ident": ident, "ls": lss})
    trace = bool(os.environ.get("BASS_TRACE"))
    if trace:
        trace = _install_profiling()
    res = run_bass_kernel_spmd(nc, in_maps, list(range(N_CORES)), trace=trace)
    LAST_RESULT = res
    out = np.concatenate([res.results[i]["out"] for i in range(N_CORES)], axis=0)
    return out


if __name__ == "__main__":
    rng = np.random.default_rng(0)
    x = rng.standard_normal((BATCH, NF), dtype=np.float32)
    cut_points = rng.random((NF, 3), dtype=np.float32)
    leaf_score = rng.random((65536, NC_OUT), dtype=np.float32)
    out = kernel(x, cut_points, leaf_score)
    print(out.shape, out.dtype, out[:2])


# revision 13
# speedup vs baseline: 1.0185x; 1.0185x over previous
"""Trainium2 Bass kernel for nn_NeuralDecisionTree.

Math (per sample b):
  h[b,f,i] = x[b,f] * W[i] + bias[f,i],   W = [1,2,3,4],
  bias[f,:] = cumsum([0, -sort(cut_points[f])])           (f=0..7, i=0..3)
  leaf[b, i0..i7] = prod_f h[b,f,i_f]                      (65536-wide kron)
  out[b,c] = sum_leaf leaf[b,leaf] * leaf_score[leaf,c]    (c=0..9)

Kernel strategy (pure batch-data-parallel over 8 cores, 256 rows each):
  W is folded into leaf_score on the host (h' = x + bias/W;
  LS' = leaf_score * kron(W,..,W)), so the device math is
  out[b,c] = sum_u A[b,u] * R[b,u,c],  R[b,:,:] = Bv[b,:] @ LSs,
  A = kron(h'0..h'2) [B,64], Bv = kron(h'3..h'7) [B,1024],
  LSs[v, c*64+u] = LS'[u*1024+v, c]  (host-prepped, fp32r-rounded, replicated).
  Per 128-row tile: kron via broadcast tensor_tensor ops (DVE/ACT),
  transpose Bv via TensorE (fp32) into packed PSUM banks, evacuate+cast to
  fp32r with wide ACT copies, contract with LSs on TensorE (fp32r, full
  1cyc/row rate when warm — dummy matmuls warm the PE clock first), final
  A-weighted segmented reduce on DVE over per-half merged PSUM.
"""

import os
import sys

sys.path.insert(0, "/opt/trn_rl_repo")

import numpy as np

import concourse.bass as bass
from concourse import bacc
import concourse.mybir as mybir
import concourse.tile as tile
from concourse.bass_utils import run_bass_kernel_spmd

F32 = mybir.dt.float32
F32R = mybir.dt.float32r

N_CORES = 8
BATCH = 2048
ROWS_PER_CORE = BATCH // N_CORES  # 256
TILES_PER_CORE = ROWS_PER_CORE // 128  # 2
NF = 8          # features
NB = 4          # bins per feature (D+1)
NC_OUT = 10     # classes
U = 64          # kron(feat 0,1,2)
V = 1024        # kron(feat 3..7)
VCHUNKS = V // 128  # 8
NCOL = NC_OUT * U   # 640 columns of LSs, layout c*64+u
NHALF = NCOL // 2   # 320 (psum per (tile,half) matmul group)
LSDMA = 4           # number of chunked ls DMAs (2 v-chunks each)
NX = TILES_PER_CORE * NF
HEADC = NX + NF * NB + 128  # head cols: x | bias | identity
NWARM = 40          # PE warm-up dummy matmuls

LAST_RESULT = None  # BassKernelResults of the most recent run (for test.py)


def _build_nc():
    nc = bacc.Bacc("TRN2", target_bir_lowering=False, debug=False,
                   num_devices=N_CORES)
    head_in = nc.declare_dram_parameter("head", [128, HEADC], F32, isOutput=False)
    ls_in = nc.declare_dram_parameter("ls", [128, VCHUNKS * NCOL], F32R, isOutput=False)
    out_ext = nc.declare_dram_parameter("out", [ROWS_PER_CORE, NC_OUT], F32, isOutput=True)

    with tile.TileContext(nc) as tc:
        with (
            tc.tile_pool(name="consts", bufs=1) as consts,
            tc.tile_pool(name="work", bufs=2) as work,
            tc.tile_pool(name="bt", bufs=2) as btp,
            tc.tile_pool(name="tpsum", bufs=3, space="PSUM") as tpsum,
            tc.tile_pool(name="rpsum", bufs=1, space="PSUM") as rpsum,
            tc.tile_pool(name="wpsum", bufs=1, space="PSUM") as wpsum,
        ):
            # PE clock warm-up: the HAM unthrottles 1.2->2.4GHz only after
            # ~3.4us of sustained matmul activity; burn that in while DMAs run.
            wt = consts.tile([128, 64], F32)
            nc.gpsimd.memset(wt[:], 0.0)
            wps = wpsum.tile([64, 64], F32)
            for _ in range(NWARM):
                nc.tensor.matmul(wps[:], wt[:], wt[:], start=True, stop=True)

            # One head DMA (x | bias | identity) issued FIRST on the same
            # HWDGE ring as the ls stream: FIFO guarantees it lands before
            # the 2.6MB ls flood instead of starving behind it.
            head = consts.tile([128, HEADC], F32)
            nc.sync.dma_start(out=head[:], in_=head_in[:])
            xa = head[:, 0:NX]
            bb = head[:, NX:NX + NF * NB]
            ident = head[:, HEADC - 128:HEADC]

            lst = []
            for j in range(LSDMA):
                lsj = consts.tile([128, (VCHUNKS // LSDMA) * NCOL], F32R, tag=f"ls{j}")
                sl = bass.ts(j, (VCHUNKS // LSDMA) * NCOL)
                nc.sync.dma_start(out=lsj[:], in_=ls_in[:, sl])
                lst.append(lsj)

            def ls_chunk(k, half):
                j, r = divmod(k, VCHUNKS // LSDMA)
                base = r * NCOL + half * NHALF
                return lst[j][:, base:base + NHALF]

            oa = consts.tile([128, TILES_PER_CORE * NC_OUT], F32)
            ab = consts.tile([128, TILES_PER_CORE * U], F32)  # A for both tiles

            def bcast0(ap, i, shape):
                return ap.unsqueeze(i).broadcast_to(shape)

            bts = []
            for t in range(TILES_PER_CORE):
                # h'[:, f*4+i] = x[:, f] + bias[f,i]/W[i]
                h = work.tile([128, NF * NB], F32, tag="h")
                nc.vector.tensor_add(
                    h[:].rearrange("p (f i) -> p f i", f=NF),
                    bcast0(xa[:, t * NF:(t + 1) * NF], 2, [128, NF, NB]),
                    bb[:].rearrange("p (f i) -> p f i", f=NF),
                )

                def hcols(f):
                    return h[:, f * NB:(f + 1) * NB]

                def kron_step(out_ap, width, prev, f):
                    # out[:, i*width+s] = prev[:, s] * h'[:, f*4+i]
                    nc.vector.tensor_mul(
                        out_ap.rearrange("p (i s) -> p i s", i=NB),
                        bcast0(prev, 1, [128, NB, width]),
                        bcast0(hcols(f), 2, [128, NB, width]),
                    )

                # Bv = kron(h3..h7): Bv[:, i3*256 + i4*64 + i5*16 + i6*4 + i7]
                b1 = work.tile([128, 16], F32, tag="b1")
                kron_step(b1[:], 4, hcols(7), 6)
                b2 = work.tile([128, 64], F32, tag="b2")
                kron_step(b2[:], 16, b1[:], 5)
                b3 = work.tile([128, 256], F32, tag="b3")
                kron_step(b3[:], 64, b2[:], 4)
                b4 = work.tile([128, V], F32, tag="b4")
                # last level split: half on DVE, half on ACT
                nc.vector.tensor_mul(
                    b4[:, 0:512].rearrange("p (i s) -> p i s", i=2),
                    bcast0(b3[:], 1, [128, 2, 256]),
                    bcast0(h[:, 3 * NB:3 * NB + 2], 2, [128, 2, 256]),
                )
                for j in range(2):
                    nc.scalar.mul(
                        b4[:, 512 + j * 256:512 + (j + 1) * 256], b3[:],
                        h[:, 3 * NB + 2 + j:3 * NB + 3 + j],
                    )

                # A = kron(h0, h1, h2): A[:, i0*16 + i1*4 + i2] (needed only
                # for the final combine, so built after the B side)
                a1 = work.tile([128, 16], F32, tag="a1")
                kron_step(a1[:], 4, hcols(2), 1)
                kron_step(ab[:, t * U:(t + 1) * U], 16, a1[:], 0)

                # Transpose Bv -> BT via TensorE (fp32); 4 chunk-transposes
                # per PSUM bank, evacuated+cast to fp32r by one wide ACT copy.
                bt = btp.tile([128, V], F32R, tag="btile")
                for q in range(2):
                    tp = tpsum.tile([128, 512], F32, tag="tp")
                    for j in range(4):
                        k = q * 4 + j
                        nc.tensor.transpose(
                            tp[:, j * 128:(j + 1) * 128],
                            b4[:, k * 128:(k + 1) * 128], ident[:],
                        )
                    nc.scalar.copy(bt[:, q * 512:(q + 1) * 512], tp[:])
                bts.append(bt)

            # R[b, c*64+u] = sum_v Bv[b,v] * LSs[v, c*64+u]  (fp32r), both
            # tiles' groups per half merged into one 2-bank psum tensor;
            # chunk-major so matmuls track the ls stream.
            pss = [rpsum.tile([128, 1024], F32, tag=f"ps{h}", name=f"ps{h}")
                   for h in range(2)]
            for k in range(VCHUNKS):
                for half in range(2):
                    for t in range(TILES_PER_CORE):
                        nc.tensor.matmul(
                            pss[half][:, t * 512:t * 512 + NHALF],
                            bts[t][:, k * 128:(k + 1) * 128],
                            ls_chunk(k, half),
                            start=(k == 0), stop=(k == VCHUNKS - 1),
                        )

            # out[b, t*10 + half*5 + c] = sum_u A[b, t*64+u] * R[...]
            abv = ab[:].rearrange("p (t u) -> p t u", t=TILES_PER_CORE)
            for half in range(2):
                tt = work.tile([128, TILES_PER_CORE * NHALF], F32, tag="tt")
                ttv = tt[:].rearrange("p (t c u) -> p t c u", t=TILES_PER_CORE, u=U)
                nc.vector.tensor_mul(
                    ttv,
                    pss[half][:].rearrange("p (t z) -> p t z", t=TILES_PER_CORE)
                        [:, :, 0:NHALF].rearrange("p t (c u) -> p t c u", u=U),
                    bcast0(abv, 2, [128, TILES_PER_CORE, NC_OUT // 2, U]),
                )
                nc.vector.reduce_sum(
                    oa[:].rearrange("p (t c) -> p t c", c=NC_OUT)
                        [:, :, half * 5:(half + 1) * 5],
                    ttv,
                    axis=mybir.AxisListType.X,
                )

            nc.scalar.dma_start(
                out=out_ext[:].rearrange("(t p) c -> p t c", p=128),
                in_=oa[:].rearrange("p (t c) -> p t c", c=NC_OUT),
            )

    nc.compile()
    return nc


_NC_CACHE = None


def _install_profiling():
    """Register the axon NTFF profile hook that this image's `antenv` lacks,
    so run_bass_kernel_spmd(trace=True) can measure HW exec time."""
    import types

    try:
        import antenv.axon_hooks  # noqa: F401
        return True
    except ImportError:
        pass
    try:
        from trn_agent_boot.trn_boot import _ntff_profile_via_ctypes
        import antenv

        hook = _ntff_profile_via_ctypes("/opt/axon/libaxon_pjrt.so")
        if hook is None:
            return False
        mod = types.ModuleType("antenv.axon_hooks")
        mod._hook = hook
        mod.set_axon_ntff_profile_hook = lambda h: setattr(mod, "_hook", h)
        mod.get_axon_ntff_profile_hook = lambda: mod._hook
        sys.modules["antenv.axon_hooks"] = mod
        antenv.axon_hooks = mod

        # Artifact upload reaches for a remote bucket; keep everything local.
        import concourse.bass_utils as bu

        bu.upload_artifacts = lambda tmpdir: "local://" + str(tmpdir)
        return True
    except Exception as e:  # pragma: no cover - best effort
        print(f"profiling hook install failed: {e!r}", file=sys.stderr)
        return False


def _to_fp32r(a):
    """Round fp32 to the PE's fp32r format: mantissa truncated to 11 bits (RNE)."""
    u = np.ascontiguousarray(np.asarray(a, np.float32)).view(np.uint32)
    low = u & np.uint32(0xFFF)
    base = u & np.uint32(0xFFFFF000)
    add = (low > 0x800) | ((low == 0x800) & (((u >> np.uint32(12)) & np.uint32(1)) == 1))
    out = base + np.where(add, np.uint32(0x1000), np.uint32(0))
    return out.view(np.float32)


def _host_prep(cut_points, leaf_score):
    W = np.arange(1.0, NB + 1.0, dtype=np.float32)               # [4]
    cp = np.sort(cut_points.astype(np.float32), axis=-1)          # [8,3]
    bias = np.cumsum(
        np.concatenate([np.zeros((NF, 1), np.float32), -cp], axis=1), axis=1
    )                                                             # [8,4]
    # W folded into leaf_score: h' = x + bias/W, LS' = LS * kron(W,...,W)
    bb = np.tile((bias / W[None, :])[None, :, :], (128, 1, 1)).reshape(128, NF * NB)
    wk = np.array([1.0], dtype=np.float64)
    for _ in range(NF):
        wk = np.kron(wk, W.astype(np.float64))                    # [65536]
    lsw = (leaf_score.astype(np.float64) * wk[:, None]).astype(np.float32)
    # LSs[p, k, c, u] = LS'[u*1024 + k*128 + p, c]
    ls4 = lsw.reshape(U, VCHUNKS, 128, NC_OUT)
    lss = np.ascontiguousarray(ls4.transpose(2, 1, 3, 0)).reshape(128, VCHUNKS * NCOL)
    lss = _to_fp32r(lss)
    ident = np.eye(128, dtype=np.float32)
    return bb, lss, ident


def _make_head(x_shard, bb, ident):
    head = np.empty((128, HEADC), dtype=np.float32)
    head[:, 0:NX] = x_shard.reshape(TILES_PER_CORE, 128, NF).transpose(1, 0, 2).reshape(128, NX)
    head[:, NX:NX + NF * NB] = bb
    head[:, HEADC - 128:HEADC] = ident
    return head


def kernel(x, cut_points, leaf_score):
    global _NC_CACHE, LAST_RESULT
    x = np.ascontiguousarray(x, dtype=np.float32)
    bb, lss, ident = _host_prep(np.asarray(cut_points), np.asarray(leaf_score))
    if _NC_CACHE is None:
        _NC_CACHE = _build_nc()
    nc = _NC_CACHE

    in_maps = []
    for i in range(N_CORES):
        xs = x[i * ROWS_PER_CORE:(i + 1) * ROWS_PER_CORE]
        in_maps.append({"head": _make_head(xs, bb, ident), "ls": lss})
    trace = bool(os.environ.get("BASS_TRACE"))
    if trace:
        trace = _install_profiling()
    res = run_bass_kernel_spmd(nc, in_maps, list(range(N_CORES)), trace=trace)
    LAST_RESULT = res
    out = np.concatenate([res.results[i]["out"] for i in range(N_CORES)], axis=0)
    return out


if __name__ == "__main__":
    rng = np.random.default_rng(0)
    x = rng.standard_normal((BATCH, NF), dtype=np.float32)
    cut_points = rng.random((NF, 3), dtype=np.float32)
    leaf_score = rng.random((65536, NC_OUT), dtype=np.float32)
    out = kernel(x, cut_points, leaf_score)
    print(out.shape, out.dtype, out[:2])
